# revision 15
# baseline (speedup 1.0000x reference)
"""Trainium2 Bass kernel for a Mamba block (nn_ATTD_MambaBlock).

Sharding: 2 (batch) x 4 (d_inner) grid over 8 NeuronCores.
Each core handles one batch element and a 384-channel slice of d_inner=1536.

Per-core pipeline (layouts are [channels-on-partitions, seqlen-on-free]):
  1. in_proj x/z as fp16 PE matmuls (K=768, 6 k-tiles), processed per l-half
     so the scan phase of half 0 overlaps phase 1 of half 1.
  2. depthwise causal conv-4 as accumulating diagonal-matrix PE matmuls
     (chunk-0 boundary handled with shortened partial matmuls).
  3. SiLU / softplus(=Ln(Exp+1)) / dA=exp(delta*A[:,n]) on ACT with
     per-partition AP scale/bias.
  4. selective scan: DVE tensor_tensor_scan per (d-tile, n), l on the free
     dim; half-1 scans chain via a carried last column (initial AP).
  5. sum over n of C_n*h_n via accumulating identity matmuls into PSUM.
  6. gating (y + x*D) * silu(z) on 4x/2x DVE ops, out_proj matmuls ->
     partial (768, L) fp32 per core; host sums the 4 d-shards per batch.
"""

import sys
import numpy as np

sys.path.insert(0, "/opt/trn_rl_repo")

import concourse.bass as bass  # noqa: E402
import concourse.tile as tile  # noqa: E402
from concourse import bacc, mybir  # noqa: E402
from contextlib import ExitStack  # noqa: E402

D_MODEL = 768
D_STATE = 16
D_CONV = 4
D_INNER = 1536
BATCH = 2
L = 2048
N_CORES = 8
D_SHARDS = 4
D_LOC = D_INNER // D_SHARDS      # 384
DT = D_LOC // 128                # 3 d-tiles of 128
KT = D_MODEL // 128              # 6 k-tiles for in_proj
MT = D_MODEL // 128              # 6 m-tiles for out_proj
H = L // 2                       # 1024, scan half

F16 = mybir.dt.float16
F32 = mybir.dt.float32
AF = mybir.ActivationFunctionType
OP = mybir.AluOpType

# packed fp32 constant columns: conv_b | w_dt | b_dt | d_vec | a_mat
C_CONVB = 0
C_WDT = 3
C_BDT = 6
C_DVEC = 9
C_AMAT = 12                      # 12 .. 12+48, dt-major: 12 + dt*16 + n
CF32_W = 64
# packed fp16 constant columns: w_x (3 k-tiles x 33) | identity | ones row
C_WX = 0                         # dt*33 .. dt*33+33
C_ID = 99
C_ONES = 227
CF16_W = 360

_PROG_CACHE = {}


def _build_program():
    nc = bacc.Bacc("TRN2", target_bir_lowering=False, debug=False,
                   num_devices=N_CORES)

    d = {}
    def di(name, shape, dtype):
        d[name] = nc.dram_tensor(name, list(shape), dtype, kind="ExternalInput").ap()

    di("hT", (128, KT, L), F16)            # hidden[b].T k-tiles: m = k*128+p
    di("w_in", (128, KT, 2 * D_LOC), F16)  # W_in shard^T k-tiles, x then z cols
    di("conv_diag", (128, DT * D_CONV * 128), F16)
    di("w_out", (128, DT, D_MODEL), F16)   # W_out shard^T k-tiles
    di("cf32", (128, CF32_W), F32)
    di("cf16", (128, CF16_W), F16)

    bc_scratch = nc.dram_tensor("bc_scratch", [2 * D_STATE, L], F16).ap()
    out_d = nc.dram_tensor("out_partial", [D_MODEL, L], F32,
                           kind="ExternalOutput").ap()

    with tile.TileContext(nc) as tc:
        with ExitStack() as ctx:
            consts = ctx.enter_context(tc.tile_pool(name="consts", bufs=1))
            big = ctx.enter_context(tc.tile_pool(name="big", bufs=1))
            hpool = ctx.enter_context(tc.tile_pool(name="hpool", bufs=1))
            psum = ctx.enter_context(tc.tile_pool(name="psum", bufs=2, space="PSUM"))
            psum_y = ctx.enter_context(tc.tile_pool(name="psum_y", bufs=1, space="PSUM"))
            scanp = ctx.enter_context(tc.tile_pool(name="scanp", bufs=3))
            bcp = ctx.enter_context(tc.tile_pool(name="bcp", bufs=3))
            t16 = ctx.enter_context(tc.tile_pool(name="t16", bufs=5))
            outp = ctx.enter_context(tc.tile_pool(name="outp", bufs=3))

            def load(name, pool=consts):
                t = pool.tile(list(d[name].shape), d[name].dtype, tag=name, name=name)
                nc.sync.dma_start(t[:], d[name][:])
                return t

            w_in = load("w_in")
            cf32 = load("cf32")
            cf16 = load("cf16")
            conv_diag = load("conv_diag")
            w_out_box = []

            # ~4.5us of dummy matmuls so the PE HAM un-throttles (cold MMs
            # run at 1.2 GHz) before the in_proj chain starts.
            warm = psum.tile([128, CF16_W], F32, tag="mm", name="warm")
            for _ in range(30):
                nc.tensor.matmul(warm[:], cf16[:, C_ID:C_ID + 128], cf16[:],
                                 start=True, stop=True, skip_group_check=True)

            warm = psum.tile([128, CF16_W], F32, tag="mm", name="warm")
            for _ in range(30):
                nc.tensor.matmul(warm[:], cf16[:, C_ID:C_ID + 128], cf16[:],
                                 start=True, stop=True, skip_group_check=True)

            def diag(dt_i, k):
                blk = (dt_i * D_CONV + k) * 128
                return conv_diag[:, blk:blk + 128]

            x_pre = [big.tile([128, L], F16, tag=f"x_pre{i}", name=f"x_pre{i}")
                     for i in range(DT)]
            x = [big.tile([128, L], F16, tag=f"x{i}", name=f"x{i}")
                 for i in range(DT)]
            sz = [big.tile([128, L], F16, tag=f"sz{i}", name=f"sz{i}")
                  for i in range(DT)]
            delta = [big.tile([128, L], F16, tag=f"delta{i}", name=f"delta{i}")
                     for i in range(DT)]
            g = [big.tile([128, L], F16, tag=f"g{i}", name=f"g{i}")
                 for i in range(DT)]
            xdbl = big.tile([33, L], F16, tag="xdbl")
            s_sb = big.tile([128, L], F16, tag="s_sb")
            carry = big.tile([128, DT * D_STATE], F32, tag="carry")

            def phase1_pieces(half):
                """Generator of phase-1 work pieces for one l-half."""
                lo = half * H

                def load_h():
                    hTh = hpool.tile([128, KT, H], F16, tag="hTh", name="hTh")
                    for k in range(KT):
                        nc.sync.dma_start(hTh[:, k, :],
                                          d["hT"][:, k, lo:lo + H])
                    return hTh

                hTh_box = []

                def in_proj(c, mlo, mhi):
                    def run():
                        if not hTh_box:
                            hTh_box.append(load_h())
                        hTh = hTh_box[0]
                        cs = lo + c * 512
                        for mi in range(mlo, mhi):
                            ps = psum.tile([128, 512], F32, tag="mm")
                            for k in range(KT):
                                nc.tensor.matmul(
                                    ps[:], w_in[:, k, mi * 128:(mi + 1) * 128],
                                    hTh[:, k, c * 512:(c + 1) * 512],
                                    start=(k == 0), stop=(k == KT - 1))
                            if mi < DT:
                                nc.scalar.copy(x_pre[mi][:, cs:cs + 512], ps[:])
                            else:
                                nc.scalar.copy(sz[mi - DT][:, cs:cs + 512], ps[:])
                    return run

                def conv_xdbl(c):
                    def run():
                        cs = lo + c * 512
                        for i in range(DT):
                            ps = psum.tile([128, 512], F32, tag="mm")
                            nc.tensor.matmul(ps[:], diag(i, 3),
                                             x_pre[i][:, cs:cs + 512],
                                             start=True, stop=False)
                            for k in (2, 1, 0):
                                sh = 3 - k
                                if cs - sh >= 0:
                                    nc.tensor.matmul(
                                        ps[:], diag(i, k),
                                        x_pre[i][:, cs - sh:cs - sh + 512],
                                        start=False, stop=(k == 0))
                                else:
                                    nc.tensor.matmul(
                                        ps[:, sh:512], diag(i, k),
                                        x_pre[i][:, 0:512 - sh],
                                        start=False, stop=(k == 0))
                            nc.scalar.activation(
                                x[i][:, cs:cs + 512], ps[:], AF.Silu,
                                bias=cf32[:, C_CONVB + i:C_CONVB + i + 1])

                        ps2 = psum.tile([33, 512], F32, tag="mm")
                        for i in range(DT):
                            nc.tensor.matmul(
                                ps2[:], cf16[:, C_WX + i * 33:C_WX + (i + 1) * 33],
                                x[i][:, cs:cs + 512],
                                start=(i == 0), stop=(i == DT - 1))
                        nc.scalar.copy(xdbl[:, cs:cs + 512], ps2[:])

                        ps3 = psum.tile([128, 512], F32, tag="mm")
                        nc.tensor.matmul(ps3[:], cf16[0:1, C_ONES:C_ONES + 128],
                                         xdbl[0:1, cs:cs + 512],
                                         start=True, stop=True)
                        nc.scalar.copy(s_sb[:, cs:cs + 512], ps3[:])
                    return run

                def tail():
                    nc.sync.dma_start(bc_scratch[:, lo:lo + H],
                                      xdbl[1:33, lo:lo + H])
                    for i in range(DT):
                        e_tmp = t16.tile([128, H], F16, tag="t16", name="e_tmp")
                        nc.scalar.activation(
                            e_tmp[:], s_sb[:, lo:lo + H], AF.Exp,
                            scale=cf32[:, C_WDT + i:C_WDT + i + 1],
                            bias=cf32[:, C_BDT + i:C_BDT + i + 1])
                        nc.scalar.activation(delta[i][:, lo:lo + H], e_tmp[:],
                                             AF.Ln, bias=1.0)
                        nc.vector.tensor_mul(g[i][:, lo:lo + H],
                                             delta[i][:, lo:lo + H],
                                             x[i][:, lo:lo + H])

                def both(a, b):
                    def run():
                        a(); b()
                    return run
                return [in_proj(0, 0, 3), conv_xdbl(0),
                        in_proj(1, 0, 3), both(conv_xdbl(1), tail),
                        in_proj(0, 3, 6), in_proj(1, 3, 6)]

            def scan_phase(half, interleave):
                """Scan phase for one half; `interleave` is a list of work
                pieces (closures) spliced between scan groups so other
                engines' streams aren't blocked behind this half's reduce."""
                lo = half * H
                ys = [psum_y.tile([128, H], F32, tag=f"ys{i}", name=f"ys{i}")
                      for i in range(DT)]
                NG = D_STATE // 2
                for grp in range(NG):
                    n0 = 2 * grp
                    bb = bcp.tile([128, 2 * H], F16, tag="bb")
                    cb = bcp.tile([128, 2 * H], F16, tag="cb")
                    for j in range(2):
                        nc.sync.dma_start(
                            bb[:, j * H:(j + 1) * H],
                            bc_scratch[n0 + j:n0 + j + 1,
                                       lo:lo + H].broadcast_to((128, H)))
                        nc.sync.dma_start(
                            cb[:, j * H:(j + 1) * H],
                            bc_scratch[D_STATE + n0 + j:D_STATE + n0 + j + 1,
                                       lo:lo + H].broadcast_to((128, H)))
                    for i in range(DT):
                        ac = C_AMAT + i * D_STATE + n0
                        dA = scanp.tile([128, 2 * H], F16, tag="dA", bufs=4)
                        nc.scalar.activation(dA[:, 0:H], delta[i][:, lo:lo + H],
                                             AF.Exp, scale=cf32[:, ac:ac + 1])
                        nc.scalar.activation(dA[:, H:2 * H],
                                             delta[i][:, lo:lo + H],
                                             AF.Exp, scale=cf32[:, ac + 1:ac + 2])
                        dBu = scanp.tile([128, 2 * H], F16, tag="dBu")
                        gv = g[i][:, lo:lo + H].unsqueeze(1).broadcast_to(
                            (128, 2, H))
                        nc.vector.tensor_tensor(
                            dBu[:].rearrange("p (b l) -> p b l", b=2), gv,
                            bb[:].rearrange("p (b l) -> p b l", b=2), OP.mult)
                        h = scanp.tile([128, 2 * H], F16, tag="h")
                        cc = i * D_STATE + n0
                        for j in range(2):
                            init = (0.0 if half == 0
                                    else carry[:, cc + j:cc + j + 1])
                            nc.vector.tensor_tensor_scan(
                                h[:, j * H:(j + 1) * H],
                                dA[:, j * H:(j + 1) * H],
                                dBu[:, j * H:(j + 1) * H], init,
                                OP.mult, OP.add)
                        if half == 0:
                            nc.scalar.copy(
                                carry[:, cc:cc + 2],
                                h[:].rearrange("p (b l) -> p b l",
                                               b=2)[:, :, H - 1])
                        hc = scanp.tile([128, 2 * H], F16, tag="hc")
                        nc.vector.tensor_mul(hc[:], h[:], cb[:])
                        for j in range(2):
                            for c in range(2):
                                nc.tensor.matmul(
                                    ys[i][:, c * 512:(c + 1) * 512],
                                    cf16[:, C_ID:C_ID + 128],
                                    hc[:, j * H + c * 512:j * H + (c + 1) * 512],
                                    start=(grp == 0 and j == 0),
                                    stop=(grp == NG - 1 and j == 1),
                                    skip_group_check=True)
                    if interleave:
                        # splice other-half work evenly between scan groups
                        want = (grp + 1) * len(interleave) // NG
                        while want > scan_phase._consumed:
                            interleave[scan_phase._consumed]()
                            scan_phase._consumed += 1
                return ys

            def gating(half, ys):
                lo = half * H
                y3h = []
                for i in range(DT):
                    y_sb = t16.tile([128, H], F16, tag="t16", name="y_sb")
                    nc.scalar.copy(y_sb[:], ys[i][:])
                    szs = t16.tile([128, H], F16, tag="t16", name="szs")
                    nc.scalar.activation(szs[:], sz[i][:, lo:lo + H], AF.Silu)
                    xd = t16.tile([128, H], F16, tag="t16", name="xd")
                    nc.vector.tensor_scalar_mul(
                        xd[:], x[i][:, lo:lo + H],
                        cf32[:, C_DVEC + i:C_DVEC + i + 1])
                    y2 = t16.tile([128, H], F16, tag="t16", name="y2")
                    nc.vector.tensor_add(y2[:], xd[:], y_sb[:])
                    y3 = scanp.tile([128, H], F16, tag=f"y3_{i}",
                                    name=f"y3_{i}", bufs=1)
                    nc.vector.tensor_mul(y3[:], y2[:], szs[:])
                    y3h.append(y3)
                return y3h

            def out_proj_pieces(half, y3h):
                lo = half * H
                pieces = []
                for mi in range(MT):
                    def piece(mi=mi):
                        if not w_out_box:
                            w_out_box.append(load("w_out"))
                        w_out = w_out_box[0]
                        for c in range(2):
                            ps = psum.tile([128, 512], F32, tag="mm")
                            for i in range(DT):
                                nc.tensor.matmul(
                                    ps[:], w_out[:, i, mi * 128:(mi + 1) * 128],
                                    y3h[i][:, c * 512:(c + 1) * 512],
                                    start=(i == 0), stop=(i == DT - 1))
                            ostage = outp.tile([128, 512], F32, tag="ostage")
                            nc.scalar.copy(ostage[:], ps[:])
                            nc.sync.dma_start(
                                out_d[mi * 128:(mi + 1) * 128,
                                      lo + c * 512:lo + (c + 1) * 512],
                                ostage[:])
                    pieces.append(piece)
                return pieces

            # ---- software pipeline across the two halves ----
            for piece in phase1_pieces(0):
                piece()
            scan_phase._consumed = 0
            ys0 = scan_phase(0, phase1_pieces(1))
            y3h0 = gating(0, ys0)
            scan_phase._consumed = 0
            ys1 = scan_phase(1, out_proj_pieces(0, y3h0))
            y3h1 = gating(1, ys1)
            for piece in out_proj_pieces(1, y3h1):
                piece()

    nc.compile()
    return nc


def _shard_inputs(inputs):
    """Build the 8 per-core input dicts (host-side layout/dtype prep)."""
    hs = np.asarray(inputs["hidden_states"], np.float32)
    W_in = np.asarray(inputs["W_in"], np.float32)
    conv_w = np.asarray(inputs["conv_w"], np.float32)
    conv_b = np.asarray(inputs["conv_b"], np.float32)
    W_x = np.asarray(inputs["W_x"], np.float32)
    W_dt = np.asarray(inputs["W_dt"], np.float32)
    b_dt = np.asarray(inputs["b_dt"], np.float32)
    A_log = np.asarray(inputs["A_log"], np.float32)
    D = np.asarray(inputs["D"], np.float32)
    W_out = np.asarray(inputs["W_out"], np.float32)

    A = -np.exp(A_log)                                   # (D_INNER, 16)
    ktile = lambda a: np.ascontiguousarray(
        a.reshape(-1, 128, a.shape[-1]).transpose(1, 0, 2))

    in_maps = []
    for core in range(N_CORES):
        b, s = divmod(core, D_SHARDS)
        d0 = s * D_LOC
        sl = slice(d0, d0 + D_LOC)
        zl = slice(D_INNER + d0, D_INNER + d0 + D_LOC)

        w_in = np.concatenate([W_in[sl].T, W_in[zl].T], 1)  # (768, 2*D_LOC)

        cw = conv_w[sl, 0, :]                            # (D_LOC, 4)
        diags = np.zeros((128, DT * D_CONV * 128), np.float16)
        for i in range(DT):
            for k in range(D_CONV):
                blk = (i * D_CONV + k) * 128
                np.fill_diagonal(diags[:, blk:blk + 128],
                                 cw[i * 128:(i + 1) * 128, k].astype(np.float16))

        pcol = lambda v: v.reshape(DT, 128).T.astype(np.float32)  # (128, DT)

        cf32 = np.zeros((128, CF32_W), np.float32)
        cf32[:, C_CONVB:C_CONVB + DT] = pcol(conv_b[sl])
        cf32[:, C_WDT:C_WDT + DT] = pcol(W_dt[sl, 0])
        cf32[:, C_BDT:C_BDT + DT] = pcol(b_dt[sl])
        cf32[:, C_DVEC:C_DVEC + DT] = pcol(D[sl])
        cf32[:, C_AMAT:C_AMAT + DT * D_STATE] = np.ascontiguousarray(
            A[sl].reshape(DT, 128, D_STATE).transpose(1, 0, 2)).reshape(128, -1)

        cf16 = np.zeros((128, CF16_W), np.float16)
        wxT = W_x[:, sl].T.astype(np.float16)            # (D_LOC, 33)
        for i in range(DT):
            cf16[:, C_WX + i * 33:C_WX + (i + 1) * 33] = wxT[i * 128:(i + 1) * 128]
        cf16[:, C_ID:C_ID + 128] = np.eye(128, dtype=np.float16)
        cf16[0, C_ONES:C_ONES + 128] = 1.0

        m = {
            "hT": ktile(hs[b].T).astype(np.float16),
            "w_in": ktile(w_in).astype(np.float16),
            "conv_diag": diags,
            "w_out": ktile(W_out[:, sl].T).astype(np.float16),
            "cf32": cf32,
            "cf16": cf16,
        }
        in_maps.append(m)
    return in_maps


def kernel(**inputs):
    from concourse.bass_utils import run_bass_kernel_spmd

    if "prog" not in _PROG_CACHE:
        _PROG_CACHE["prog"] = _build_program()
    nc = _PROG_CACHE["prog"]

    in_maps = _shard_inputs(inputs)
    res = run_bass_kernel_spmd(nc, in_maps, core_ids=list(range(N_CORES)),
                               **_PROG_CACHE.get("run_kwargs", {}))
    _PROG_CACHE["last_result"] = res

    out = np.zeros((BATCH, L, D_MODEL), np.float32)
    for b in range(BATCH):
        acc = np.zeros((D_MODEL, L), np.float32)
        for s in range(D_SHARDS):
            acc += res.results[b * D_SHARDS + s]["out_partial"]
        out[b] = acc.T
    return out
